# revision 20
# baseline (speedup 1.0000x reference)
"""Bahdanau attention TRN2 kernel.

B=32, S=2048, H=U=1024, fp32. Data-parallel over batch: 4 batches per
NeuronCore across 8 cores. Per core, per batch:
  keysT[u,s] = W1[:,u].T @ hiddensT[:,s]  (fp32r matmuls, h contracted)
  t = tanh(keysT + q[u] + b1[u])          (ACT, bias per partition)
  score[1,s] = V.T @ t                    (accumulating fp32r matmuls)
  attn = softmax(score)                   (shift-free: exp(score+bV-m), m=sum|V|+|bV|)
  ctx[1,h] = attnT.T @ hiddens_nat        (fp32r matmuls, s contracted)
hiddensT tiles come from PE transposes of the naturally-loaded (and
fp32r-rounded during DMA) hiddens chunks, which stay resident in SBUF for
the context pass, so HBM traffic is one read of hiddens.
"""

import os
import sys

sys.path.insert(0, "/opt/trn_rl_repo")
# The NTFF profiling hook (antenv.axon_hooks) is not available in this
# container; force-disable tracing so a stray BASS_TRACE doesn't break runs.
os.environ["BASS_NEVER_TRACE"] = "1"

from contextlib import ExitStack

import numpy as np

import concourse.bacc as bacc
import concourse.tile as tile
from concourse import mybir
from concourse.bass_utils import run_bass_kernel_spmd
from concourse.masks import make_identity

F32 = mybir.dt.float32
F32R = mybir.dt.float32r
AF = mybir.ActivationFunctionType

B, S, H, U = 32, 2048, 1024, 1024
NCORES = 8
BPC = B // NCORES          # batches per core
NCH = 4                    # s-chunks per batch
SC = S // NCH              # 512 s per chunk
NS = SC // 128             # 4 s-subtiles per chunk
KH = H // 128              # 8 h-tiles
KU = U // 128              # 8 u-tiles
NUC = U // 512             # 2 u-chunks of 512


def build_nc(reps=1):
    nc = bacc.Bacc("TRN2", target_bir_lowering=False, debug=False)

    hid_d = nc.dram_tensor("hiddens", [BPC, S, H], F32, kind="ExternalInput")
    ht_d = nc.dram_tensor("hidden_t", [BPC, H], F32, kind="ExternalInput")
    w1_d = nc.dram_tensor("W1", [H, U], F32, kind="ExternalInput")
    b1_d = nc.dram_tensor("b1", [U], F32, kind="ExternalInput")
    w2_d = nc.dram_tensor("W2", [H, U], F32, kind="ExternalInput")
    b2_d = nc.dram_tensor("b2", [U], F32, kind="ExternalInput")
    v_d = nc.dram_tensor("V", [U, 1], F32, kind="ExternalInput")
    bv_d = nc.dram_tensor("bV", [1], F32, kind="ExternalInput")
    ctx_d = nc.dram_tensor("ctx_out", [BPC, H], F32, kind="ExternalOutput")
    attn_d = nc.dram_tensor("attn_out", [BPC, S], F32, kind="ExternalOutput")

    with tile.TileContext(nc) as tc, ExitStack() as ctx:
        wts = ctx.enter_context(tc.tile_pool(name="wts", bufs=1))
        htp = ctx.enter_context(tc.tile_pool(name="htp", bufs=2))
        natp = ctx.enter_context(tc.tile_pool(name="natp", bufs=3))
        tp = ctx.enter_context(tc.tile_pool(name="tp", bufs=3))
        w2s = ctx.enter_context(tc.tile_pool(name="w2s", bufs=3))
        sm = ctx.enter_context(tc.tile_pool(name="sm", bufs=1))
        smb = ctx.enter_context(tc.tile_pool(name="smb", bufs=2))
        ps = ctx.enter_context(tc.tile_pool(name="ps", bufs=2, space="PSUM"))

        # ---------------- constants ----------------
        identf = sm.tile([128, 128], F32)
        make_identity(nc, identf[:])
        identr = sm.tile([128, 128], F32R)
        nc.vector.tensor_copy(identr[:], identf[:])

        # first hiddens chunk DMA issued ahead of the weight loads
        _nat0 = natp.tile([128, NS * H], F32R, tag="nat", name="nat_first")
        nc.gpsimd.dma_start(
            _nat0[:].rearrange("p (si h) -> p si h", si=NS),
            hid_d[0, 0:SC, :].rearrange("(si p) h -> p si h", p=128),
        )

        # W1 as fp32r, laid out [h mod 128, (h_tile, u)]
        w1r = wts.tile([128, KH * U], F32R)
        nc.gpsimd.dma_start(
            w1r[:].rearrange("p (k u) -> p k u", k=KH),
            w1_d[:, :].rearrange("(k p) u -> p k u", p=128),
        )

        # V: [U,1] -> [1,U], transpose to [u mod 128, u_tile] via K=1 matmuls
        vr = sm.tile([1, U], F32)
        nc.sync.dma_start(vr[:], v_d[:, :].rearrange("u one -> one u"))
        tpv = ps.tile([128, KU], F32, tag="tp")
        for k in range(KU):
            nc.tensor.matmul(
                tpv[:, k : k + 1],
                vr[0:1, 128 * k : 128 * (k + 1)],
                identf[0:1, 0:1],
                start=True,
                stop=True,
            )
        v_sb = sm.tile([128, KU], F32R)
        nc.vector.tensor_copy(v_sb[:], tpv[:])

        # m = sum(|V|) + |bV|; exp bias eb = bV - m
        bv_sb = sm.tile([1, 1], F32)
        nc.sync.dma_start(bv_sb[:], bv_d[:].rearrange("(a o) -> a o", a=1))
        absv = sm.tile([1, U], F32)
        sv = sm.tile([1, 1], F32)
        nc.scalar.activation(absv[:], vr[:], AF.Abs, accum_out=sv[:])
        absbv = sm.tile([1, 1], F32)
        nc.scalar.activation(absbv[:], bv_sb[:], AF.Abs)
        mtot = sm.tile([1, 1], F32)
        nc.vector.tensor_add(mtot[:], sv[:], absbv[:])
        eb = sm.tile([1, 1], F32)
        nc.vector.tensor_sub(eb[:], bv_sb[:], mtot[:])

        # bbT[u mod 128, u_tile] = (b1 + b2) transposed
        b1_sb = sm.tile([1, U], F32)
        nc.sync.dma_start(b1_sb[:], b1_d[:].rearrange("(a u) -> a u", a=1))
        b2_sb = sm.tile([1, U], F32)
        nc.sync.dma_start(b2_sb[:], b2_d[:].rearrange("(a u) -> a u", a=1))
        b12 = sm.tile([1, U], F32)
        nc.vector.tensor_add(b12[:], b1_sb[:], b2_sb[:])
        tpb = ps.tile([128, KU], F32, tag="tp")
        for k in range(KU):
            nc.tensor.matmul(
                tpb[:, k : k + 1],
                b12[0:1, 128 * k : 128 * (k + 1)],
                identf[0:1, 0:1],
                start=True,
                stop=True,
            )
        bbt = sm.tile([128, KU], F32)
        nc.vector.tensor_copy(bbt[:], tpb[:])

        # hidden_t [BPC, H] -> htT [h mod 128, (h_tile, b)] fp32r
        htr = sm.tile([BPC, H], F32)
        nc.sync.dma_start(htr[:], ht_d[:, :])
        tph = ps.tile([128, KH * BPC], F32, tag="tp")
        for k in range(KH):
            nc.tensor.matmul(
                tph[:, BPC * k : BPC * (k + 1)],
                htr[0:BPC, 128 * k : 128 * (k + 1)],
                identf[0:BPC, 0:BPC],
                start=True,
                stop=True,
            )
        htt = sm.tile([128, KH * BPC], F32R)
        nc.vector.tensor_copy(htt[:], tph[:])

        # q[b,u] = hidden_t[b] @ W2 + b2 (+b1): accumulate over h tiles with
        # W2 streamed in [128,512] fp32r chunks; result transposed into
        # qb[u mod 128, (u_tile, b)] with bias added.
        qb = sm.tile([128, KU * BPC], F32)
        qsb = sm.tile([BPC, U], F32R)
        for uc in range(NUC):
            qps = ps.tile([BPC, 512], F32, tag="sc")
            for k in range(KH):
                w2c = w2s.tile([128, 512], F32R)
                nc.gpsimd.dma_start(
                    w2c[:], w2_d[128 * k : 128 * (k + 1), 512 * uc : 512 * (uc + 1)]
                )
                nc.tensor.matmul(
                    qps[:],
                    htt[:, BPC * k : BPC * (k + 1)],
                    w2c[:],
                    start=(k == 0),
                    stop=(k == KH - 1),
                )
            nc.vector.tensor_copy(qsb[:, 512 * uc : 512 * (uc + 1)], qps[:])
        for ub in range(KU):
            tpq = ps.tile([128, BPC], F32R, tag="tp")
            nc.tensor.transpose(
                tpq[:],
                qsb[0:BPC, 128 * ub : 128 * (ub + 1)],
                identr[0:BPC, 0:BPC],
            )
            nc.vector.tensor_scalar_add(
                qb[:, BPC * ub : BPC * (ub + 1)],
                tpq[:].bitcast(F32),
                bbt[:, ub : ub + 1],
            )

        # ---------------- main loop ----------------
        # Per chunk c: load+transpose, keys/tanh/score matmuls, exp, then (one
        # chunk later) transpose the exp chunk and accumulate unnormalized
        # context matmuls; 1/Z is applied to the context at batch end. This
        # keeps PE busy across batch boundaries and frees nat tiles per-chunk.
        nats = {}    # (b, c) -> nat tile
        esbs = {}    # (r, b) -> exp(score - m) [1, S]
        zparts = {}  # (r, b) -> per-chunk exp sums [1, NCH]
        ats = {}     # b -> at_sb [128, 16] fp32r (exp transposed)
        cxs = {}     # b -> [cxps_j for j] psum tiles [1, 512]

        def emit_load(r, b, c):
            nat = natp.tile([128, NS * H], F32R, tag="nat", name=f"nat{r}_{b}_{c}")
            nats[(b, c)] = nat
            nc.gpsimd.dma_start(
                nat[:].rearrange("p (si h) -> p si h", si=NS),
                hid_d[b, SC * c : SC * (c + 1), :].rearrange(
                    "(si p) h -> p si h", p=128
                ),
            )
            htc = htp.tile([128, KH * SC], F32R, tag="ht", name=f"ht{r}_{b}_{c}")
            return nat, htc

        def emit_tr_group(nat, htc, k):
            tpt = ps.tile([128, SC], F32R, tag="tp")
            for si in range(NS):
                nc.tensor.transpose(
                    tpt[:, 128 * si : 128 * (si + 1)],
                    nat[:, H * si + 128 * k : H * si + 128 * (k + 1)],
                    identr[:],
                )
            nc.vector.tensor_copy(htc[:, SC * k : SC * (k + 1)], tpt[:])

        def emit_compute(r, b, c, htc, nxt=None, tail_cb=None):
            scps = ps.tile([1, SC], F32, tag="sc")
            if (r, b) not in esbs:
                esbs[(r, b)] = smb.tile([1, S], F32, tag="esb", name=f"esb{r}_{b}")
                zparts[(r, b)] = smb.tile([1, NCH], F32, tag="zp", name=f"zp{r}_{b}")
            for ut in range(KU):
                kps = ps.tile([128, SC], F32, tag="kps")
                for k in range(KH):
                    nc.tensor.matmul(
                        kps[:],
                        w1r[:, U * k + 128 * ut : U * k + 128 * (ut + 1)],
                        htc[:, SC * k : SC * (k + 1)],
                        start=(k == 0),
                        stop=(k == KH - 1),
                    )
                tt = tp.tile([128, SC], F32R, tag="t")
                nc.scalar.activation(
                    tt[:], kps[:], AF.Tanh, bias=qb[:, BPC * ut + b : BPC * ut + b + 1]
                )
                nc.tensor.matmul(
                    scps[:],
                    v_sb[:, ut : ut + 1],
                    tt[:],
                    start=(ut == 0),
                    stop=(ut == KU - 1),
                )
                if ut == 0 and tail_cb is not None:
                    tail_cb()
                if nxt is not None:
                    emit_tr_group(nxt[0], nxt[1], ut)
            nc.scalar.activation(
                esbs[(r, b)][:, SC * c : SC * (c + 1)],
                scps[:],
                AF.Exp,
                bias=eb[0:1, 0:1],
                accum_out=zparts[(r, b)][:, c : c + 1],
            )

        def emit_tail(r, b, c):
            # transpose exp chunk c into at_sb columns, then accumulate the
            # unnormalized context for this chunk
            if b not in ats:
                ats[b] = smb.tile([128, S // 128], F32R, tag="at", name=f"at{r}_{b}")
                cxs[b] = [
                    ps.tile([1, 512], F32, tag="cx", name=f"cx{r}_{b}_{j}")
                    for j in range(H // 512)
                ]
            tpa = ps.tile([128, NS], F32, tag="tp")
            for si in range(NS):
                nc.tensor.matmul(
                    tpa[:, si : si + 1],
                    esbs[(r, b)][0:1, SC * c + 128 * si : SC * c + 128 * (si + 1)],
                    identf[0:1, 0:1],
                    start=True,
                    stop=True,
                )
            nc.vector.tensor_copy(ats[b][:, NS * c : NS * (c + 1)], tpa[:])
            for j in range(H // 512):
                for si in range(NS):
                    nc.tensor.matmul(
                        cxs[b][j],
                        ats[b][:, NS * c + si : NS * c + si + 1],
                        nats[(b, c)][:, H * si + 512 * j : H * si + 512 * (j + 1)],
                        start=(c == 0 and si == 0),
                        stop=(c == NCH - 1 and si == NS - 1),
                    )
            del nats[(b, c)]

        def emit_batch_tail(r, b):
            # softmax normalizer, attn output, context scale + output
            zsum = smb.tile([1, 1], F32, tag="zs", name=f"zs{r}_{b}")
            nc.vector.reduce_sum(zsum[:], zparts[(r, b)][:], axis=mybir.AxisListType.X)
            rz = smb.tile([1, 1], F32, tag="rz", name=f"rz{r}_{b}")
            nc.vector.reciprocal(rz[:], zsum[:])
            attn = smb.tile([1, S], F32, tag="attn", name=f"attn{r}_{b}")
            nc.vector.tensor_scalar_mul(attn[:], esbs[(r, b)][:], rz[0:1, 0:1])
            nc.sync.dma_start(attn_d[b, :].rearrange("(a s) -> a s", a=1), attn[:])
            ctx_sb = smb.tile([1, H], F32, tag="ctx", name=f"ctx{r}_{b}")
            for j in range(H // 512):
                nc.vector.tensor_scalar_mul(
                    ctx_sb[:, 512 * j : 512 * (j + 1)], cxs[b][j], rz[0:1, 0:1]
                )
            del cxs[b], ats[b]
            nc.sync.dma_start(ctx_d[b, :].rearrange("(a h) -> a h", a=1), ctx_sb[:])

        chunks = [
            (r, b, c) for r in range(reps) for b in range(BPC) for c in range(NCH)
        ]
        loaded = {}
        _htc0 = htp.tile([128, KH * SC], F32R, tag="ht", name="ht_first")
        nats[(0, 0)] = _nat0
        first_load = (_nat0, _htc0)
        loaded[chunks[0]] = first_load  # DMA was issued before the weight DMAs
        for k in range(KH):
            emit_tr_group(first_load[0], first_load[1], k)
        prev = None
        for i, (r, b, c) in enumerate(chunks):
            cur = loaded.pop((r, b, c))
            if i + 1 < len(chunks):
                loaded[chunks[i + 1]] = emit_load(*chunks[i + 1])
                nxt = loaded[chunks[i + 1]]
            else:
                nxt = None
            if prev is not None:
                pr, pb, pc = prev

                def tail_cb(pr=pr, pb=pb, pc=pc):
                    emit_tail(pr, pb, pc)
                    if pc == NCH - 1:
                        emit_batch_tail(pr, pb)
            else:
                tail_cb = None
            emit_compute(r, b, c, cur[1], nxt=nxt, tail_cb=tail_cb)
            prev = (r, b, c)
        emit_tail(*prev)
        emit_batch_tail(prev[0], prev[1])

    nc.compile()
    return nc


_NC = None


def _get_nc():
    global _NC
    if _NC is None:
        _NC = build_nc()
    return _NC


def kernel(hidden_t, hiddens, W1, b1, W2, b2, V, bV):
    hidden_t = np.ascontiguousarray(np.asarray(hidden_t, dtype=np.float32))
    hiddens = np.ascontiguousarray(np.asarray(hiddens, dtype=np.float32))
    W1 = np.ascontiguousarray(np.asarray(W1, dtype=np.float32))
    b1 = np.ascontiguousarray(np.asarray(b1, dtype=np.float32))
    W2 = np.ascontiguousarray(np.asarray(W2, dtype=np.float32))
    b2 = np.ascontiguousarray(np.asarray(b2, dtype=np.float32))
    V = np.ascontiguousarray(np.asarray(V, dtype=np.float32))
    bV = np.ascontiguousarray(np.asarray(bV, dtype=np.float32))

    nc = _get_nc()
    in_maps = []
    for i in range(NCORES):
        sl = slice(i * BPC, (i + 1) * BPC)
        in_maps.append(
            {
                "hiddens": np.ascontiguousarray(hiddens[sl]),
                "hidden_t": np.ascontiguousarray(hidden_t[sl]),
                "W1": W1,
                "b1": b1,
                "W2": W2,
                "b2": b2,
                "V": V,
                "bV": bV,
            }
        )
    res = run_bass_kernel_spmd(nc, in_maps, core_ids=list(range(NCORES)))
    ctxv = np.concatenate([r["ctx_out"] for r in res.results], axis=0)
    attn = np.concatenate([r["attn_out"] for r in res.results], axis=0)[..., None]
    return ctxv, attn


# revision 25
# speedup vs baseline: 1.4199x; 1.4199x over previous
"""Bahdanau attention TRN2 kernel.

B=32, S=2048, H=U=1024, fp32. Data-parallel over batch: 4 batches per
NeuronCore across 8 cores. Per core, per batch:
  keysT[u,s] = W1[:,u].T @ hiddensT[:,s]  (fp32r matmuls, h contracted)
  t = tanh(keysT + q[u] + b1[u])          (ACT, bias per partition)
  score[1,s] = V.T @ t                    (accumulating fp32r matmuls)
  attn = softmax(score)                   (shift-free: exp(score+bV-m), m=sum|V|+|bV|)
  ctx[1,h] = attnT.T @ hiddens_nat        (fp32r matmuls, s contracted)
hiddensT tiles come from PE transposes of the naturally-loaded (and
fp32r-rounded during DMA) hiddens chunks, which stay resident in SBUF for
the context pass, so HBM traffic is one read of hiddens.
"""

import os
import sys

sys.path.insert(0, "/opt/trn_rl_repo")
# The NTFF profiling hook (antenv.axon_hooks) is not available in this
# container; force-disable tracing so a stray BASS_TRACE doesn't break runs.
os.environ["BASS_NEVER_TRACE"] = "1"

from contextlib import ExitStack

import numpy as np

import concourse.bacc as bacc
import concourse.tile as tile
from concourse import mybir
from concourse.bass_utils import run_bass_kernel_spmd
from concourse.masks import make_identity

F32 = mybir.dt.float32
F32R = mybir.dt.float32r
AF = mybir.ActivationFunctionType

B, S, H, U = 32, 2048, 1024, 1024
NCORES = 8
BPC = B // NCORES          # batches per core
NCH = 4                    # s-chunks per batch
SC = S // NCH              # 512 s per chunk
NS = SC // 128             # 4 s-subtiles per chunk
KH = H // 128              # 8 h-tiles
KU = U // 128              # 8 u-tiles
NUC = U // 512             # 2 u-chunks of 512


def build_nc(reps=1, skip=frozenset()):
    nc = bacc.Bacc("TRN2", target_bir_lowering=False, debug=False)

    hid_d = nc.dram_tensor("hiddens", [BPC, S, H], F32, kind="ExternalInput")
    ht_d = nc.dram_tensor("hidden_t", [BPC, H], F32, kind="ExternalInput")
    w1_d = nc.dram_tensor("W1", [H, U], F32, kind="ExternalInput")
    b1_d = nc.dram_tensor("b1", [U], F32, kind="ExternalInput")
    w2_d = nc.dram_tensor("W2", [H, U], F32, kind="ExternalInput")
    b2_d = nc.dram_tensor("b2", [U], F32, kind="ExternalInput")
    v_d = nc.dram_tensor("V", [U, 1], F32, kind="ExternalInput")
    bv_d = nc.dram_tensor("bV", [1], F32, kind="ExternalInput")
    ctx_d = nc.dram_tensor("ctx_out", [BPC, H], F32, kind="ExternalOutput")
    attn_d = nc.dram_tensor("attn_out", [BPC, S], F32, kind="ExternalOutput")

    with tile.TileContext(nc) as tc, ExitStack() as ctx:
        wts = ctx.enter_context(tc.tile_pool(name="wts", bufs=1))
        htp = ctx.enter_context(tc.tile_pool(name="htp", bufs=2))
        natp = ctx.enter_context(tc.tile_pool(name="natp", bufs=3))
        tp = ctx.enter_context(tc.tile_pool(name="tp", bufs=3))
        w2s = ctx.enter_context(tc.tile_pool(name="w2s", bufs=3))
        sm = ctx.enter_context(tc.tile_pool(name="sm", bufs=1))
        smb = ctx.enter_context(tc.tile_pool(name="smb", bufs=2))
        ps = ctx.enter_context(tc.tile_pool(name="ps", bufs=2, space="PSUM"))

        # ---------------- constants ----------------
        identf = sm.tile([128, 128], F32)
        make_identity(nc, identf[:])
        identr = sm.tile([128, 128], F32R)
        nc.vector.tensor_copy(identr[:], identf[:])

        # first hiddens chunk DMA issued ahead of the weight loads
        _nat0 = natp.tile([128, NS * H], F32R, tag="nat", name="nat_first")
        nc.gpsimd.dma_start(
            _nat0[:].rearrange("p (si h) -> p si h", si=NS),
            hid_d[0, 0:SC, :].rearrange("(si p) h -> p si h", p=128),
        )

        # W1 as fp32r, laid out [h mod 128, (h_tile, u)]
        w1r = wts.tile([128, KH * U], F32R)
        nc.gpsimd.dma_start(
            w1r[:].rearrange("p (k u) -> p k u", k=KH),
            w1_d[:, :].rearrange("(k p) u -> p k u", p=128),
        )

        # V: [U,1] -> [1,U], transpose to [u mod 128, u_tile] via K=1 matmuls
        vr = sm.tile([1, U], F32)
        nc.sync.dma_start(vr[:], v_d[:, :].rearrange("u one -> one u"))
        tpv = ps.tile([128, KU], F32, tag="tp")
        for k in range(KU):
            nc.tensor.matmul(
                tpv[:, k : k + 1],
                vr[0:1, 128 * k : 128 * (k + 1)],
                identf[0:1, 0:1],
                start=True,
                stop=True,
            )
        v_sb = sm.tile([128, KU], F32R)
        nc.vector.tensor_copy(v_sb[:], tpv[:])

        # m = sum(|V|) + |bV|; exp bias eb = bV - m
        bv_sb = sm.tile([1, 1], F32)
        nc.sync.dma_start(bv_sb[:], bv_d[:].rearrange("(a o) -> a o", a=1))
        absv = sm.tile([1, U], F32)
        sv = sm.tile([1, 1], F32)
        nc.scalar.activation(absv[:], vr[:], AF.Abs, accum_out=sv[:])
        absbv = sm.tile([1, 1], F32)
        nc.scalar.activation(absbv[:], bv_sb[:], AF.Abs)
        mtot = sm.tile([1, 1], F32)
        nc.vector.tensor_add(mtot[:], sv[:], absbv[:])
        eb = sm.tile([1, 1], F32)
        nc.vector.tensor_sub(eb[:], bv_sb[:], mtot[:])

        # bbT[u mod 128, u_tile] = (b1 + b2) transposed
        b1_sb = sm.tile([1, U], F32)
        nc.sync.dma_start(b1_sb[:], b1_d[:].rearrange("(a u) -> a u", a=1))
        b2_sb = sm.tile([1, U], F32)
        nc.sync.dma_start(b2_sb[:], b2_d[:].rearrange("(a u) -> a u", a=1))
        b12 = sm.tile([1, U], F32)
        nc.vector.tensor_add(b12[:], b1_sb[:], b2_sb[:])
        tpb = ps.tile([128, KU], F32, tag="tp")
        for k in range(KU):
            nc.tensor.matmul(
                tpb[:, k : k + 1],
                b12[0:1, 128 * k : 128 * (k + 1)],
                identf[0:1, 0:1],
                start=True,
                stop=True,
            )
        bbt = sm.tile([128, KU], F32)
        nc.vector.tensor_copy(bbt[:], tpb[:])

        # hidden_t [BPC, H] -> htT [h mod 128, (h_tile, b)] fp32r
        htr = sm.tile([BPC, H], F32)
        nc.sync.dma_start(htr[:], ht_d[:, :])
        tph = ps.tile([128, KH * BPC], F32, tag="tp")
        for k in range(KH):
            nc.tensor.matmul(
                tph[:, BPC * k : BPC * (k + 1)],
                htr[0:BPC, 128 * k : 128 * (k + 1)],
                identf[0:BPC, 0:BPC],
                start=True,
                stop=True,
            )
        htt = sm.tile([128, KH * BPC], F32R)
        nc.vector.tensor_copy(htt[:], tph[:])

        # q[b,u] = hidden_t[b] @ W2 + b2 (+b1): accumulate over h tiles with
        # W2 streamed in [128,512] fp32r chunks; result transposed into
        # qb[u mod 128, (u_tile, b)] with bias added.
        qb = sm.tile([128, KU * BPC], F32)
        qsb = sm.tile([BPC, U], F32R)
        for uc in range(NUC):
            qps = ps.tile([BPC, 512], F32, tag="sc")
            for k in range(KH):
                w2c = w2s.tile([128, 512], F32R)
                nc.gpsimd.dma_start(
                    w2c[:], w2_d[128 * k : 128 * (k + 1), 512 * uc : 512 * (uc + 1)]
                )
                nc.tensor.matmul(
                    qps[:],
                    htt[:, BPC * k : BPC * (k + 1)],
                    w2c[:],
                    start=(k == 0),
                    stop=(k == KH - 1),
                )
            nc.vector.tensor_copy(qsb[:, 512 * uc : 512 * (uc + 1)], qps[:])
        for ub in range(KU):
            tpq = ps.tile([128, BPC], F32R, tag="tp")
            nc.tensor.transpose(
                tpq[:],
                qsb[0:BPC, 128 * ub : 128 * (ub + 1)],
                identr[0:BPC, 0:BPC],
            )
            nc.vector.tensor_scalar_add(
                qb[:, BPC * ub : BPC * (ub + 1)],
                tpq[:].bitcast(F32),
                bbt[:, ub : ub + 1],
            )

        # ---------------- main loop ----------------
        # Per chunk c: load+transpose, keys/tanh/score matmuls, exp, then (one
        # chunk later) transpose the exp chunk and accumulate unnormalized
        # context matmuls; 1/Z is applied to the context at batch end. This
        # keeps PE busy across batch boundaries and frees nat tiles per-chunk.
        nats = {}    # (b, c) -> nat tile
        esbs = {}    # (r, b) -> exp(score - m) [1, S]
        zparts = {}  # (r, b) -> per-chunk exp sums [1, NCH]
        ats = {}     # b -> at_sb [128, 16] fp32r (exp transposed)
        cxs = {}     # b -> [cxps_j for j] psum tiles [1, 512]

        def emit_load(r, b, c):
            nat = natp.tile([128, NS * H], F32R, tag="nat", name=f"nat{r}_{b}_{c}")
            nats[(b, c)] = nat
            nc.gpsimd.dma_start(
                nat[:].rearrange("p (si h) -> p si h", si=NS),
                hid_d[b, SC * c : SC * (c + 1), :].rearrange(
                    "(si p) h -> p si h", p=128
                ),
            )
            htc = htp.tile([128, KH * SC], F32R, tag="ht", name=f"ht{r}_{b}_{c}")
            return nat, htc

        def emit_tr_group(nat, htc, k):
            tpt = ps.tile([128, SC], F32R, tag="tp")
            for si in range(NS):
                if "tr4" in skip and si > 0:
                    continue
                nc.tensor.transpose(
                    tpt[:, 128 * si : 128 * (si + 1)],
                    nat[:, H * si + 128 * k : H * si + 128 * (k + 1)],
                    identr[:],
                )
            if "cpact" in skip:
                nc.scalar.copy(htc[:, SC * k : SC * (k + 1)], tpt[:])
            else:
                nc.vector.tensor_copy(htc[:, SC * k : SC * (k + 1)], tpt[:])

        def emit_compute(r, b, c, htc, nxt=None, tail_cb=None):
            scps = ps.tile([1, SC], F32, tag="sc")
            if (r, b) not in esbs:
                esbs[(r, b)] = smb.tile([1, S], F32, tag="esb", name=f"esb{r}_{b}")
                zparts[(r, b)] = smb.tile([1, NCH], F32, tag="zp", name=f"zp{r}_{b}")
            for ut in range(KU):
                kps = ps.tile([128, SC], F32, tag="kps")
                for k in range(KH):
                    if "keys1" in skip and 0 < k < KH - 1:
                        continue
                    nc.tensor.matmul(
                        kps[:],
                        w1r[:, U * k + 128 * ut : U * k + 128 * (ut + 1)],
                        htc[:, SC * k : SC * (k + 1)],
                        start=(k == 0),
                        stop=(k == KH - 1),
                    )
                tt = tp.tile([128, SC], F32R, tag="t")
                nc.scalar.activation(
                    tt[:], kps[:], AF.Tanh, bias=qb[:, BPC * ut + b : BPC * ut + b + 1]
                )
                nc.tensor.matmul(
                    scps[:],
                    v_sb[:, ut : ut + 1],
                    tt[:],
                    start=(ut == 0),
                    stop=(ut == KU - 1),
                )
                if ut == 0 and tail_cb is not None:
                    tail_cb()
                if nxt is not None and "inter" in skip:
                    emit_tr_group(nxt[0], nxt[1], ut)
                if ut == 3 and nxt is not None and "inter" not in skip and "early" not in skip:
                    for kk in range(KH):
                        emit_tr_group(nxt[0], nxt[1], kk)
            nc.scalar.activation(
                esbs[(r, b)][:, SC * c : SC * (c + 1)],
                scps[:],
                AF.Exp,
                bias=eb[0:1, 0:1],
                accum_out=zparts[(r, b)][:, c : c + 1],
            )

        def emit_tail(r, b, c):
            # transpose exp chunk c into at_sb columns, then accumulate the
            # unnormalized context for this chunk
            if b not in ats:
                ats[b] = smb.tile([128, S // 128], F32R, tag="at", name=f"at{r}_{b}")
                cxs[b] = [
                    ps.tile([1, 512], F32, tag="cx", name=f"cx{r}_{b}_{j}")
                    for j in range(H // 512)
                ]
            tpa = ps.tile([128, NS], F32, tag="tp")
            for si in range(NS):
                nc.tensor.matmul(
                    tpa[:, si : si + 1],
                    esbs[(r, b)][0:1, SC * c + 128 * si : SC * c + 128 * (si + 1)],
                    identf[0:1, 0:1],
                    start=True,
                    stop=True,
                )
            nc.vector.tensor_copy(ats[b][:, NS * c : NS * (c + 1)], tpa[:])
            for j in range(H // 512):
                for si in range(NS):
                    nc.tensor.matmul(
                        cxs[b][j],
                        ats[b][:, NS * c + si : NS * c + si + 1],
                        nats[(b, c)][:, H * si + 512 * j : H * si + 512 * (j + 1)],
                        start=(c == 0 and si == 0),
                        stop=(c == NCH - 1 and si == NS - 1),
                    )
            del nats[(b, c)]

        def emit_batch_tail(r, b):
            # softmax normalizer, attn output, context scale + output
            zsum = smb.tile([1, 1], F32, tag="zs", name=f"zs{r}_{b}")
            nc.vector.reduce_sum(zsum[:], zparts[(r, b)][:], axis=mybir.AxisListType.X)
            rz = smb.tile([1, 1], F32, tag="rz", name=f"rz{r}_{b}")
            nc.vector.reciprocal(rz[:], zsum[:])
            attn = smb.tile([1, S], F32, tag="attn", name=f"attn{r}_{b}")
            nc.vector.tensor_scalar_mul(attn[:], esbs[(r, b)][:], rz[0:1, 0:1])
            nc.sync.dma_start(attn_d[b, :].rearrange("(a s) -> a s", a=1), attn[:])
            ctx_sb = smb.tile([1, H], F32, tag="ctx", name=f"ctx{r}_{b}")
            for j in range(H // 512):
                nc.vector.tensor_scalar_mul(
                    ctx_sb[:, 512 * j : 512 * (j + 1)], cxs[b][j], rz[0:1, 0:1]
                )
            del cxs[b], ats[b]
            nc.sync.dma_start(ctx_d[b, :].rearrange("(a h) -> a h", a=1), ctx_sb[:])

        chunks = [
            (r, b, c) for r in range(reps) for b in range(BPC) for c in range(NCH)
        ]
        loaded = {}
        _htc0 = htp.tile([128, KH * SC], F32R, tag="ht", name="ht_first")
        nats[(0, 0)] = _nat0
        first_load = (_nat0, _htc0)
        loaded[chunks[0]] = first_load  # DMA was issued before the weight DMAs
        for k in range(KH):
            emit_tr_group(first_load[0], first_load[1], k)
        prev = None
        for i, (r, b, c) in enumerate(chunks):
            cur = loaded.pop((r, b, c))
            if i + 1 < len(chunks):
                loaded[chunks[i + 1]] = emit_load(*chunks[i + 1])
                nxt = loaded[chunks[i + 1]]
            else:
                nxt = None
            if prev is not None:
                pr, pb, pc = prev

                def tail_cb(pr=pr, pb=pb, pc=pc):
                    emit_tail(pr, pb, pc)
                    if pc == NCH - 1:
                        emit_batch_tail(pr, pb)
            else:
                tail_cb = None
            if "early" in skip and nxt is not None:
                for k in range(KH):
                    emit_tr_group(nxt[0], nxt[1], k)
            emit_compute(r, b, c, cur[1], nxt=nxt, tail_cb=tail_cb)
            prev = (r, b, c)
        emit_tail(*prev)
        emit_batch_tail(prev[0], prev[1])

    nc.compile()
    return nc


_NC = None


def _get_nc():
    global _NC
    if _NC is None:
        _NC = build_nc()
    return _NC


def kernel(hidden_t, hiddens, W1, b1, W2, b2, V, bV):
    hidden_t = np.ascontiguousarray(np.asarray(hidden_t, dtype=np.float32))
    hiddens = np.ascontiguousarray(np.asarray(hiddens, dtype=np.float32))
    W1 = np.ascontiguousarray(np.asarray(W1, dtype=np.float32))
    b1 = np.ascontiguousarray(np.asarray(b1, dtype=np.float32))
    W2 = np.ascontiguousarray(np.asarray(W2, dtype=np.float32))
    b2 = np.ascontiguousarray(np.asarray(b2, dtype=np.float32))
    V = np.ascontiguousarray(np.asarray(V, dtype=np.float32))
    bV = np.ascontiguousarray(np.asarray(bV, dtype=np.float32))

    nc = _get_nc()
    in_maps = []
    for i in range(NCORES):
        sl = slice(i * BPC, (i + 1) * BPC)
        in_maps.append(
            {
                "hiddens": np.ascontiguousarray(hiddens[sl]),
                "hidden_t": np.ascontiguousarray(hidden_t[sl]),
                "W1": W1,
                "b1": b1,
                "W2": W2,
                "b2": b2,
                "V": V,
                "bV": bV,
            }
        )
    res = run_bass_kernel_spmd(nc, in_maps, core_ids=list(range(NCORES)))
    ctxv = np.concatenate([r["ctx_out"] for r in res.results], axis=0)
    attn = np.concatenate([r["attn_out"] for r in res.results], axis=0)[..., None]
    return ctxv, attn
